# revision 12
# baseline (speedup 1.0000x reference)
"""Trainium2 Bass kernel for nn_EncoderTransformer_61194694033513.

Data-parallel over batch B=16 across 8 NeuronCores (2 batch elems per core).
Per core, the whole forward runs out of SBUF with activations stored
feature-major HT[e, tok] in fp16 (matmul operands must be 16-bit to stream at
1 column/cycle on the PE; fp32 matmul runs at 1/4 rate; fp16 carries 10
mantissa bits vs bf16's 7, and squares are pre-scaled by 1/64 to stay in
fp16 range). All matmul
accumulation is fp32 in PSUM. Attention is computed flash-style (S^T tiles of
[128 keys x 512 queries], relu, accumulated into O^T) so the [N,N] matrix is
never materialized. LayerNorm reductions over the feature (partition) axis go
through the PE with a ones lhsT into [1,512] PSUM rows; the per-token row math
runs in fp32 on partition 0, rstd is computed as exp(-0.5*ln(var+eps)) on the
scalar engine (one table set, no slow DVE reciprocal), and rstd / mean*rstd
rows are broadcast back over partitions with K=1 matmuls whose lhsT carries
g / -g. The apply is one tensor_tensor + one scalar_tensor_tensor per 128x512
block (beta rides in as the per-partition fp32 scalar).
"""

import sys

import numpy as np

for _p in (
    "/opt/trn_rl_repo",
    "/opt/pypackages",
    "/root/.axon_site",
    "/root/.axon_site/_ro/trn_rl_repo",
    "/root/.axon_site/_ro/pypackages",
):
    if _p not in sys.path:
        sys.path.append(_p)

import ml_dtypes  # noqa: E402

import concourse.bass as bass  # noqa: E402
import concourse.bacc as bacc  # noqa: E402
import concourse.mybir as mybir  # noqa: E402
from concourse import tile  # noqa: E402
from concourse.bass_utils import run_bass_kernel_spmd  # noqa: E402

B, N, D, E, L = 16, 2048, 128, 256, 3
NCORES = 8
BL = B // NCORES  # batch elems per core
P = 128
EC = E // P  # feature-dim partition chunks (2)
CH = N // 512  # 512-wide token chunks (4)
JT = N // P  # key tiles (16)
EPS = 1e-5
F32 = mybir.dt.float32
F16 = mybir.dt.float16
NPF16 = np.float16
AF = mybir.ActivationFunctionType
OP = mybir.AluOpType

_CACHE = {}


def _build():
    nc = bacc.Bacc("TRN2", target_bir_lowering=False, debug=False, num_devices=NCORES)

    d_xsT = nc.declare_dram_parameter("xsT", [BL, P, N], F16, isOutput=False)
    d_Win = nc.declare_dram_parameter("Win", [D, E], F16, isOutput=False)
    d_W = {
        nm: nc.declare_dram_parameter(nm, [L, E, E], F16, isOutput=False)
        for nm in ("Wq", "Wk", "Wv", "W1", "W2")
    }
    d_WoutT = nc.declare_dram_parameter("WoutT", [E, N], F32, isOutput=False)
    d_binp = nc.declare_dram_parameter("binp", [P, EC], F32, isOutput=False)
    d_bm1 = nc.declare_dram_parameter("bm1p", [L, P, EC], F32, isOutput=False)
    d_bm2 = nc.declare_dram_parameter("bm2p", [L, P, EC], F32, isOutput=False)
    d_be1 = nc.declare_dram_parameter("be1p", [L, P, EC], F32, isOutput=False)
    d_be2 = nc.declare_dram_parameter("be2p", [L, P, EC], F32, isOutput=False)
    # g rows for the LN broadcast matmuls: [L, EC, 2, P]; row 0 = g, row 1 = -g
    d_ln1 = nc.declare_dram_parameter("ln1rows", [L, EC, 2, P], F16, isOutput=False)
    d_ln2 = nc.declare_dram_parameter("ln2rows", [L, EC, 2, P], F16, isOutput=False)
    d_bout = nc.declare_dram_parameter("b_out", [1, 1], F32, isOutput=False)
    d_out = nc.declare_dram_parameter("out", [BL, 1], F32, isOutput=True)

    with tile.TileContext(nc) as tc:
        from contextlib import ExitStack

        with ExitStack() as ctx:
            cpool = ctx.enter_context(tc.tile_pool(name="const", bufs=1))
            hpool = ctx.enter_context(tc.tile_pool(name="acts", bufs=1))
            xs_pool = ctx.enter_context(tc.tile_pool(name="xs", bufs=2))
            spool = ctx.enter_context(tc.tile_pool(name="srelu", bufs=6))
            sqpool = ctx.enter_context(tc.tile_pool(name="sqp", bufs=6))
            apool = ctx.enter_context(tc.tile_pool(name="mlpa", bufs=4))
            tpool = ctx.enter_context(tc.tile_pool(name="t1p", bufs=4))
            ropool = ctx.enter_context(tc.tile_pool(name="ro", bufs=2))
            wopool = ctx.enter_context(tc.tile_pool(name="wo", bufs=2))

            PS = bass.MemorySpace.PSUM
            ps_s = ctx.enter_context(tc.tile_pool(name="ps_s", bufs=2, space=PS))
            ps_o = ctx.enter_context(tc.tile_pool(name="ps_o", bufs=2, space=PS))
            ps_mm = ctx.enter_context(tc.tile_pool(name="ps_mm", bufs=2, space=PS))

            # ---- constants / weights -------------------------------------
            w_sb = {}
            for nm in ("Wq", "Wk", "Wv", "W1", "W2"):
                w_sb[nm] = []
                for l in range(L):
                    tl = []
                    for ec in range(EC):
                        t = cpool.tile([P, E], F16, name=f"{nm}{l}{ec}", tag=f"{nm}{l}{ec}")
                        nc.sync.dma_start(t[:], d_W[nm][l, ec * P : (ec + 1) * P, :])
                        tl.append(t)
                    w_sb[nm].append(tl)
            win_sb = cpool.tile([P, E], F16, name="win", tag="win")
            nc.sync.dma_start(win_sb[:], d_Win[:])

            def col_param(dram, nm, per_l=True):
                out = []
                for l in range(L if per_l else 1):
                    t = cpool.tile([P, EC], F32, name=f"{nm}{l}", tag=f"{nm}{l}")
                    nc.sync.dma_start(t[:], dram[l] if per_l else dram[:])
                    out.append(t)
                return out

            binp_sb = col_param(d_binp, "binp", per_l=False)[0]
            bm1_sb = col_param(d_bm1, "bm1")
            bm2_sb = col_param(d_bm2, "bm2")
            be1_sb = col_param(d_be1, "be1")
            be2_sb = col_param(d_be2, "be2")

            # g / -g broadcast rows: [1,128] bf16 tiles per (ln, l, pt)
            lnrow_sb = {1: [], 2: []}
            for which, dram in ((1, d_ln1), (2, d_ln2)):
                for l in range(L):
                    per_pt = []
                    for pt in range(EC):
                        rows = []
                        for r in range(2):
                            t = cpool.tile(
                                [1, P], F16,
                                name=f"ln{which}_{l}{pt}{r}", tag=f"ln{which}_{l}{pt}{r}",
                            )
                            nc.sync.dma_start(t[:], dram[l, pt, r : r + 1, :])
                            rows.append(t)
                        per_pt.append(rows)
                    lnrow_sb[which].append(per_pt)
            bout_sb = cpool.tile([1, 1], F32, name="bout", tag="bout")
            nc.sync.dma_start(bout_sb[:], d_bout[:])

            ones_kb = cpool.tile([P, 1], F16, name="ones_kb", tag="ones_kb")
            nc.vector.memset(ones_kb[:], 1.0)
            ones_kf = cpool.tile([P, 1], F32, name="ones_kf", tag="ones_kf")
            nc.vector.memset(ones_kf[:], 1.0)
            eps1 = cpool.tile([1, 1], F32, name="eps1", tag="eps1")
            nc.vector.memset(eps1[:], EPS)

            # LN row scratch: partition 0. rowsF fp32 (sum / sumsq / var),
            # rowsB bf16 (rstd / mean*rstd) for the broadcast matmul rhs.
            rowsF = cpool.tile([1, 3 * N], F32, name="rowsF", tag="rowsF")
            rowsB = cpool.tile([1, 2 * N], F16, name="rowsB", tag="rowsB")
            s0 = rowsF[:, 0:N]
            s1 = rowsF[:, N : 2 * N]
            xr = rowsF[:, 2 * N : 3 * N]
            rstd_row = rowsB[:, 0:N]
            mrstd_row = rowsB[:, N : 2 * N]

            # ---- persistent activations (fp16), one set per batch elem ----
            Hf = [[hpool.tile([P, N], F16, name=f"Hf{b}{ec}", tag=f"Hf{b}{ec}") for ec in range(EC)] for b in range(BL)]
            qT = [[hpool.tile([P, N], F16, name=f"qT{b}{dc}", tag=f"qT{b}{dc}") for dc in range(EC)] for b in range(BL)]
            kT = [[hpool.tile([P, N], F16, name=f"kT{b}{dc}", tag=f"kT{b}{dc}") for dc in range(EC)] for b in range(BL)]
            v_sb = [hpool.tile([P, JT * E], F16, name=f"v{b}", tag=f"v{b}") for b in range(BL)]

            def layernorm(X, rows_lpt, be_col):
                """In-place LN over the feature axis of X (list of 2 [P,N] bf16
                tiles). rows_lpt[pt] = (g_row, negg_row); be_col[:, pt] = beta."""
                for c in range(CH):
                    cs = slice(c * 512, (c + 1) * 512)
                    sqc = []
                    for pt in range(EC):
                        sq = sqpool.tile([P, 512], F16, name="sq", tag="sq")
                        # (x/64)^2 on the otherwise-idle GpSimd (no fused
                        # stt opcode on Pool): scale then square
                        sqt = sqpool.tile([P, 512], F16, name="sqt", tag="sqt")
                        nc.gpsimd.tensor_scalar_mul(sqt[:], X[pt][:, cs], 1.0 / 64)
                        nc.gpsimd.tensor_mul(sq[:], sqt[:], sqt[:])
                        sqc.append(sq)
                    st_s = ps_mm.tile([1, 512], F32, name="st_s", tag="mm")
                    nc.tensor.matmul(st_s[:], ones_kb[:], X[0][:, cs], start=True, stop=False)
                    nc.tensor.matmul(st_s[:], ones_kb[:], X[1][:, cs], start=False, stop=True)
                    st_q = ps_mm.tile([1, 512], F32, name="st_q", tag="mm")
                    nc.tensor.matmul(st_q[:], ones_kb[:], sqc[0][:], start=True, stop=False)
                    nc.tensor.matmul(st_q[:], ones_kb[:], sqc[1][:], start=False, stop=True)
                    nc.scalar.copy(rowsF[:, c * 512 : (c + 1) * 512], st_s[:])
                    nc.scalar.copy(rowsF[:, N + c * 512 : N + (c + 1) * 512], st_q[:])
                # row math on partition 0 (fp32)
                nc.vector.tensor_mul(xr, s0, s0)  # s0^2
                nc.vector.scalar_tensor_tensor(
                    xr, xr, -1.0 / (E * 4096.0), s1, op0=OP.mult, op1=OP.add
                )  # (E*var)/4096 = s1 - s0^2/(E*4096)
                # rstd = 1/sqrt(|var + eps|) in one ACT op; abs_reciprocal_sqrt
                # shares its table set with relu/square/identity/copy, so the
                # kernel needs a single ACT table load.
                nc.scalar.activation(
                    rstd_row, xr, AF.Abs_reciprocal_sqrt, bias=eps1[:], scale=4096.0 / E
                )
                nc.vector.scalar_tensor_tensor(
                    mrstd_row, s0, 1.0 / E, rstd_row, op0=OP.mult, op1=OP.mult
                )  # mean * rstd
                for pt in range(EC):
                    grow, ngrow = rows_lpt[pt]
                    for c2 in range(CH // 2):
                        cs2 = slice(c2 * 1024, (c2 + 1) * 1024)
                        rb = ps_s.tile([P, 1024], F32, name="rb", tag="s")
                        cb = ps_s.tile([P, 1024], F32, name="cb", tag="s")
                        for h in range(2):
                            cs = slice(c2 * 1024 + h * 512, c2 * 1024 + (h + 1) * 512)
                            hs = slice(h * 512, (h + 1) * 512)
                            nc.tensor.matmul(rb[:, hs], grow[:], rstd_row[:, cs])
                            nc.tensor.matmul(cb[:, hs], ngrow[:], mrstd_row[:, cs])
                        t1 = tpool.tile([P, 1024], F16, name="t1", tag="t1")
                        nc.vector.tensor_mul(t1[:], X[pt][:, cs2], rb[:])
                        # X = (t1 + be) + cb  = x*g*rstd - g*mean*rstd + be
                        nc.vector.scalar_tensor_tensor(
                            X[pt][:, cs2], t1[:], be_col[:, pt : pt + 1], cb[:],
                            op0=OP.add, op1=OP.add,
                        )

            # ---- per batch element, layers interleaved across the two batch
            # elems so one stream's LN row math overlaps the other's matmuls --
            def input_proj(b):
                xs_t = xs_pool.tile([P, N], F16, name="xs", tag="xs")
                nc.sync.dma_start(xs_t[:], d_xsT[b])
                for ec in range(EC):
                    es = slice(ec * P, (ec + 1) * P)
                    for c in range(CH):
                        cs = slice(c * 512, (c + 1) * 512)
                        ps = ps_mm.tile([P, 512], F32, name="psin", tag="mm")
                        nc.tensor.matmul(ps[:], win_sb[:, es], xs_t[:, cs])
                        nc.vector.tensor_scalar_add(Hf[b][ec][:, cs], ps[:], binp_sb[:, ec : ec + 1])

            def qkv(b, l):
                for w_name, dstT in (("Wq", qT[b]), ("Wk", kT[b])):
                    for dc in range(EC):
                        ds_ = slice(dc * P, (dc + 1) * P)
                        for c in range(CH):
                            cs = slice(c * 512, (c + 1) * 512)
                            ps = ps_mm.tile([P, 512], F32, name="psqk", tag="mm")
                            for ec in range(EC):
                                nc.tensor.matmul(
                                    ps[:],
                                    w_sb[w_name][l][ec][:, ds_],
                                    Hf[b][ec][:, cs],
                                    start=(ec == 0),
                                    stop=(ec == EC - 1),
                                )
                            if w_name == "Wq":
                                nc.scalar.copy(dstT[dc][:, cs], ps[:])
                            else:
                                nc.vector.tensor_copy(dstT[dc][:, cs], ps[:])
                for t in range(JT):
                    ps = ps_mm.tile([P, E], F32, name="psv", tag="mm")
                    for ec in range(EC):
                        nc.tensor.matmul(
                            ps[:],
                            Hf[b][ec][:, t * P : (t + 1) * P],
                            w_sb["Wv"][l][ec][:],
                            start=(ec == 0),
                            stop=(ec == EC - 1),
                        )
                    if t % 2 == 0:
                        nc.scalar.copy(v_sb[b][:, t * E : (t + 1) * E], ps[:])
                    else:
                        nc.vector.tensor_copy(v_sb[b][:, t * E : (t + 1) * E], ps[:])

            def attention(b):
                for c in range(CH):
                    cs = slice(c * 512, (c + 1) * 512)
                    o_ps = [
                        ps_o.tile([P, 512], F32, name=f"o{oc}", tag="o")
                        for oc in range(EC)
                    ]
                    for j2 in range(JT // 2):
                        s_ps = ps_s.tile([P, 1024], F32, name="s_ps", tag="s")
                        for h in range(2):
                            j = 2 * j2 + h
                            hs = slice(h * 512, (h + 1) * 512)
                            for dc in range(EC):
                                nc.tensor.matmul(
                                    s_ps[:, hs],
                                    kT[b][dc][:, j * P : (j + 1) * P],
                                    qT[b][dc][:, cs],
                                    start=(dc == 0),
                                    stop=(dc == EC - 1),
                                )
                        sr = spool.tile([P, 1024], F16, name="sr", tag="sr")
                        nc.scalar.activation(sr[:], s_ps[:], AF.Relu)
                        for h in range(2):
                            j = 2 * j2 + h
                            hs = slice(h * 512, (h + 1) * 512)
                            for oc in range(EC):
                                nc.tensor.matmul(
                                    o_ps[oc][:],
                                    v_sb[b][:, j * E + oc * P : j * E + (oc + 1) * P],
                                    sr[:, hs],
                                    start=(j == 0),
                                    stop=(j == JT - 1),
                                )
                    for oc in range(EC):
                        nc.vector.tensor_add(Hf[b][oc][:, cs], Hf[b][oc][:, cs], o_ps[oc][:])

            def mlp(b, l):
                for c in range(CH):
                    cs = slice(c * 512, (c + 1) * 512)
                    a_t = []
                    for mc in range(EC):
                        ms = slice(mc * P, (mc + 1) * P)
                        ps = ps_mm.tile([P, 512], F32, name="psa", tag="mm")
                        for ec in range(EC):
                            nc.tensor.matmul(
                                ps[:],
                                w_sb["W1"][l][ec][:, ms],
                                Hf[b][ec][:, cs],
                                start=(ec == 0),
                                stop=(ec == EC - 1),
                            )
                        a = apool.tile([P, 512], F16, name="a", tag="a")
                        nc.scalar.activation(
                            a[:], ps[:], AF.Relu, bias=bm1_sb[l][:, mc : mc + 1]
                        )
                        a_t.append(a)
                    for oc in range(EC):
                        os_ = slice(oc * P, (oc + 1) * P)
                        ps = ps_mm.tile([P, 512], F32, name="psm", tag="mm")
                        for mc in range(EC):
                            nc.tensor.matmul(
                                ps[:],
                                w_sb["W2"][l][mc][:, os_],
                                a_t[mc][:],
                                start=(mc == 0),
                                stop=(mc == EC - 1),
                            )
                        nc.vector.scalar_tensor_tensor(
                            Hf[b][oc][:, cs],
                            ps[:],
                            bm2_sb[l][:, oc : oc + 1],
                            Hf[b][oc][:, cs],
                            op0=OP.add,
                            op1=OP.add,
                        )

            def readout(b):
                r8 = ropool.tile([P, 2 * CH], F32, name="r8", tag="r8")
                idx = 0
                for ec in range(EC):
                    for c in range(CH):
                        cs = slice(c * 512, (c + 1) * 512)
                        wo = wopool.tile([P, 512], F32, name="wo", tag="wo")
                        nc.sync.dma_start(
                            wo[:], d_WoutT[ec * P : (ec + 1) * P, c * 512 : (c + 1) * 512]
                        )
                        ros = ropool.tile([P, 512], F32, name="ros", tag="ros")
                        nc.vector.tensor_mul(ros[:], Hf[b][ec][:, cs], wo[:])
                        nc.vector.reduce_sum(
                            r8[:, idx : idx + 1], ros[:], axis=mybir.AxisListType.X
                        )
                        idx += 1
                rsum = ropool.tile([P, 1], F32, name="rsum", tag="rsum")
                nc.vector.reduce_sum(rsum[:], r8[:], axis=mybir.AxisListType.X)
                ps = ps_mm.tile([1, 1], F32, name="psro", tag="mm")
                nc.tensor.matmul(ps[:], ones_kf[:], rsum[:])
                ob = ropool.tile([1, 1], F32, name="ob", tag="ob")
                nc.scalar.activation(ob[:], ps[:], AF.Identity, bias=bout_sb[:])
                nc.sync.dma_start(d_out[b : b + 1, :], ob[:])

            for b in range(BL):
                input_proj(b)
            for l in range(L):
                for b in range(BL):
                    qkv(b, l)
                    attention(b)
                    layernorm(Hf[b], lnrow_sb[1][l], be1_sb[l])
                    mlp(b, l)
                    layernorm(Hf[b], lnrow_sb[2][l], be2_sb[l])
            for b in range(BL):
                readout(b)

    nc.compile()
    return nc


def _prep_inputs(inputs):
    f = lambda x: np.asarray(x, np.float32)
    bf = lambda x: np.ascontiguousarray(np.asarray(x, np.float32).astype(NPF16))
    xs = f(inputs["xs"])
    xsT = np.ascontiguousarray(xs.transpose(0, 2, 1)).astype(NPF16)  # [B, D, N]
    WoutT = np.ascontiguousarray(f(inputs["Wout"]).reshape(N, E).T)  # [E, N]

    def cols(v, per_l):
        v = f(v)
        if per_l:
            return np.ascontiguousarray(v.reshape(L, EC, P).transpose(0, 2, 1))
        return np.ascontiguousarray(v.reshape(EC, P).T)

    def grows(g):
        g = f(g).reshape(L, EC, P)
        return np.ascontiguousarray(np.stack([g, -g], axis=2)).astype(NPF16)

    common = {
        "Win": bf(inputs["Win"]),
        "Wq": bf(inputs["Wq"]),
        "Wk": bf(inputs["Wk"]),
        "Wv": bf(inputs["Wv"]),
        "W1": bf(inputs["W1"]),
        "W2": bf(inputs["W2"]),
        "WoutT": WoutT,
        "binp": cols(inputs["b_in"], False),
        "bm1p": cols(inputs["bm1"], True),
        "bm2p": cols(inputs["bm2"], True),
        "be1p": cols(inputs["be1"], True),
        "be2p": cols(inputs["be2"], True),
        "ln1rows": grows(inputs["g1"]),
        "ln2rows": grows(inputs["g2"]),
        "b_out": f(inputs["b_out"]).reshape(1, 1),
    }
    in_maps = []
    for c in range(NCORES):
        m = dict(common)
        m["xsT"] = np.ascontiguousarray(xsT[c * BL : (c + 1) * BL])
        in_maps.append(m)
    return in_maps


def get_program():
    if "nc" not in _CACHE:
        _CACHE["nc"] = _build()
    return _CACHE["nc"]


def kernel(**inputs) -> np.ndarray:
    nc = get_program()
    in_maps = _prep_inputs(inputs)
    res = run_bass_kernel_spmd(nc, in_maps, list(range(NCORES)))
    out = np.concatenate([res.results[c]["out"] for c in range(NCORES)], axis=0)
    return out.astype(np.float32)


# revision 13
# speedup vs baseline: 1.7639x; 1.7639x over previous
"""Trainium2 Bass kernel for nn_EncoderTransformer_61194694033513.

Data-parallel over batch B=16 across 8 NeuronCores (2 batch elems per core).
Per core, the whole forward runs out of SBUF with activations stored
feature-major HT[e, tok] in fp16 (matmul operands must be 16-bit to stream at
1 column/cycle on the PE; fp32 matmul runs at 1/4 rate; fp16 carries 10
mantissa bits vs bf16's 7, and squares are pre-scaled by 1/64 to stay in
fp16 range). All matmul
accumulation is fp32 in PSUM. Attention is computed flash-style (S^T tiles of
[128 keys x 512 queries], relu, accumulated into O^T) so the [N,N] matrix is
never materialized. LayerNorm reductions over the feature (partition) axis go
through the PE with a ones lhsT into [1,512] PSUM rows; the per-token row math
runs in fp32 on partition 0, rstd is computed as exp(-0.5*ln(var+eps)) on the
scalar engine (one table set, no slow DVE reciprocal), and rstd / mean*rstd
rows are broadcast back over partitions with K=1 matmuls whose lhsT carries
g / -g. The apply is one tensor_tensor + one scalar_tensor_tensor per 128x512
block (beta rides in as the per-partition fp32 scalar).
"""

import sys

import numpy as np

for _p in (
    "/opt/trn_rl_repo",
    "/opt/pypackages",
    "/root/.axon_site",
    "/root/.axon_site/_ro/trn_rl_repo",
    "/root/.axon_site/_ro/pypackages",
):
    if _p not in sys.path:
        sys.path.append(_p)

import ml_dtypes  # noqa: E402

import concourse.bass as bass  # noqa: E402
import concourse.bacc as bacc  # noqa: E402
import concourse.mybir as mybir  # noqa: E402
from concourse import tile  # noqa: E402
from concourse.bass_utils import run_bass_kernel_spmd  # noqa: E402

B, N, D, E, L = 16, 2048, 128, 256, 3
NCORES = 8
BL = B // NCORES  # batch elems per core
P = 128
EC = E // P  # feature-dim partition chunks (2)
CH = N // 512  # 512-wide token chunks (4)
JT = N // P  # key tiles (16)
EPS = 1e-5
F32 = mybir.dt.float32
F16 = mybir.dt.float16
NPF16 = np.float16
AF = mybir.ActivationFunctionType
OP = mybir.AluOpType

_CACHE = {}


def _build():
    nc = bacc.Bacc("TRN2", target_bir_lowering=False, debug=False, num_devices=NCORES)

    d_xsT = nc.declare_dram_parameter("xsT", [BL, P, N], F16, isOutput=False)
    d_Win = nc.declare_dram_parameter("Win", [D, E], F16, isOutput=False)
    d_W = {
        nm: nc.declare_dram_parameter(nm, [L, E, E], F16, isOutput=False)
        for nm in ("Wq", "Wk", "Wv", "W1", "W2")
    }
    d_WoutT = nc.declare_dram_parameter("WoutT", [E, N], F32, isOutput=False)
    d_binp = nc.declare_dram_parameter("binp", [P, EC], F32, isOutput=False)
    d_bm1 = nc.declare_dram_parameter("bm1p", [L, P, EC], F32, isOutput=False)
    d_bm2 = nc.declare_dram_parameter("bm2p", [L, P, EC], F32, isOutput=False)
    d_be1 = nc.declare_dram_parameter("be1p", [L, P, EC], F32, isOutput=False)
    d_be2 = nc.declare_dram_parameter("be2p", [L, P, EC], F32, isOutput=False)
    # g rows for the LN broadcast matmuls: [L, EC, 2, P]; row 0 = g, row 1 = -g
    d_ln1 = nc.declare_dram_parameter("ln1rows", [L, EC, 2, P], F16, isOutput=False)
    d_ln2 = nc.declare_dram_parameter("ln2rows", [L, EC, 2, P], F16, isOutput=False)
    d_bout = nc.declare_dram_parameter("b_out", [1, 1], F32, isOutput=False)
    d_out = nc.declare_dram_parameter("out", [BL, 1], F32, isOutput=True)

    with tile.TileContext(nc) as tc:
        from contextlib import ExitStack

        with ExitStack() as ctx:
            cpool = ctx.enter_context(tc.tile_pool(name="const", bufs=1))
            hpool = ctx.enter_context(tc.tile_pool(name="acts", bufs=1))
            xs_pool = ctx.enter_context(tc.tile_pool(name="xs", bufs=2))
            spool = ctx.enter_context(tc.tile_pool(name="srelu", bufs=6))
            sqpool = ctx.enter_context(tc.tile_pool(name="sqp", bufs=6))
            apool = ctx.enter_context(tc.tile_pool(name="mlpa", bufs=4))
            tpool = ctx.enter_context(tc.tile_pool(name="t1p", bufs=4))
            ropool = ctx.enter_context(tc.tile_pool(name="ro", bufs=2))
            wopool = ctx.enter_context(tc.tile_pool(name="wo", bufs=2))

            PS = bass.MemorySpace.PSUM
            ps_s = ctx.enter_context(tc.tile_pool(name="ps_s", bufs=2, space=PS))
            ps_o = ctx.enter_context(tc.tile_pool(name="ps_o", bufs=2, space=PS))
            ps_mm = ctx.enter_context(tc.tile_pool(name="ps_mm", bufs=2, space=PS))

            # ---- constants / weights -------------------------------------
            w_sb = {}
            for nm in ("Wq", "Wk", "Wv", "W1", "W2"):
                w_sb[nm] = []
                for l in range(L):
                    tl = []
                    for ec in range(EC):
                        t = cpool.tile([P, E], F16, name=f"{nm}{l}{ec}", tag=f"{nm}{l}{ec}")
                        nc.sync.dma_start(t[:], d_W[nm][l, ec * P : (ec + 1) * P, :])
                        tl.append(t)
                    w_sb[nm].append(tl)
            win_sb = cpool.tile([P, E], F16, name="win", tag="win")
            nc.sync.dma_start(win_sb[:], d_Win[:])

            def col_param(dram, nm, per_l=True):
                out = []
                for l in range(L if per_l else 1):
                    t = cpool.tile([P, EC], F32, name=f"{nm}{l}", tag=f"{nm}{l}")
                    nc.sync.dma_start(t[:], dram[l] if per_l else dram[:])
                    out.append(t)
                return out

            binp_sb = col_param(d_binp, "binp", per_l=False)[0]
            bm1_sb = col_param(d_bm1, "bm1")
            bm2_sb = col_param(d_bm2, "bm2")
            be1_sb = col_param(d_be1, "be1")
            be2_sb = col_param(d_be2, "be2")

            # g / -g broadcast rows: [1,128] bf16 tiles per (ln, l, pt)
            lnrow_sb = {1: [], 2: []}
            for which, dram in ((1, d_ln1), (2, d_ln2)):
                for l in range(L):
                    per_pt = []
                    for pt in range(EC):
                        rows = []
                        for r in range(2):
                            t = cpool.tile(
                                [1, P], F16,
                                name=f"ln{which}_{l}{pt}{r}", tag=f"ln{which}_{l}{pt}{r}",
                            )
                            nc.sync.dma_start(t[:], dram[l, pt, r : r + 1, :])
                            rows.append(t)
                        per_pt.append(rows)
                    lnrow_sb[which].append(per_pt)
            bout_sb = cpool.tile([1, 1], F32, name="bout", tag="bout")
            nc.sync.dma_start(bout_sb[:], d_bout[:])

            ones_kb = cpool.tile([P, 1], F16, name="ones_kb", tag="ones_kb")
            nc.vector.memset(ones_kb[:], 1.0)
            ones_kf = cpool.tile([P, 1], F32, name="ones_kf", tag="ones_kf")
            nc.vector.memset(ones_kf[:], 1.0)
            eps1 = cpool.tile([1, 1], F32, name="eps1", tag="eps1")
            nc.vector.memset(eps1[:], EPS)

            # LN row scratch: partition 0. rowsF fp32 (sum / sumsq / var),
            # rowsB bf16 (rstd / mean*rstd) for the broadcast matmul rhs.
            rowsF = cpool.tile([1, 3 * N], F32, name="rowsF", tag="rowsF")
            rowsB = cpool.tile([1, 2 * N], F16, name="rowsB", tag="rowsB")
            s0 = rowsF[:, 0:N]
            s1 = rowsF[:, N : 2 * N]
            xr = rowsF[:, 2 * N : 3 * N]
            rstd_row = rowsB[:, 0:N]
            mrstd_row = rowsB[:, N : 2 * N]

            # ---- persistent activations (fp16), one set per batch elem ----
            Hf = [[hpool.tile([P, N], F16, name=f"Hf{b}{ec}", tag=f"Hf{b}{ec}") for ec in range(EC)] for b in range(BL)]
            qT = [[hpool.tile([P, N], F16, name=f"qT{b}{dc}", tag=f"qT{b}{dc}") for dc in range(EC)] for b in range(BL)]
            kT = [[hpool.tile([P, N], F16, name=f"kT{b}{dc}", tag=f"kT{b}{dc}") for dc in range(EC)] for b in range(BL)]
            v_sb = [hpool.tile([P, JT * E], F16, name=f"v{b}", tag=f"v{b}") for b in range(BL)]

            def layernorm(X, rows_lpt, be_col):
                """In-place LN over the feature axis of X (list of 2 [P,N] bf16
                tiles). rows_lpt[pt] = (g_row, negg_row); be_col[:, pt] = beta."""
                for c in range(CH):
                    cs = slice(c * 512, (c + 1) * 512)
                    sqc = []
                    for pt in range(EC):
                        sq = sqpool.tile([P, 512], F16, name="sq", tag="sq")
                        nc.scalar.activation(sq[:], X[pt][:, cs], AF.Square, scale=1.0 / 64)
                        sqc.append(sq)
                    st_s = ps_mm.tile([1, 512], F32, name="st_s", tag="mm")
                    nc.tensor.matmul(st_s[:], ones_kb[:], X[0][:, cs], start=True, stop=False)
                    nc.tensor.matmul(st_s[:], ones_kb[:], X[1][:, cs], start=False, stop=True)
                    st_q = ps_mm.tile([1, 512], F32, name="st_q", tag="mm")
                    nc.tensor.matmul(st_q[:], ones_kb[:], sqc[0][:], start=True, stop=False)
                    nc.tensor.matmul(st_q[:], ones_kb[:], sqc[1][:], start=False, stop=True)
                    nc.scalar.copy(rowsF[:, c * 512 : (c + 1) * 512], st_s[:])
                    nc.scalar.copy(rowsF[:, N + c * 512 : N + (c + 1) * 512], st_q[:])
                # row math on partition 0 (fp32)
                nc.vector.tensor_mul(xr, s0, s0)  # s0^2
                nc.vector.scalar_tensor_tensor(
                    xr, xr, -1.0 / (E * 4096.0), s1, op0=OP.mult, op1=OP.add
                )  # (E*var)/4096 = s1 - s0^2/(E*4096)
                # rstd = 1/sqrt(|var + eps|) in one ACT op; abs_reciprocal_sqrt
                # shares its table set with relu/square/identity/copy, so the
                # kernel needs a single ACT table load.
                nc.scalar.activation(
                    rstd_row, xr, AF.Abs_reciprocal_sqrt, bias=eps1[:], scale=4096.0 / E
                )
                nc.vector.scalar_tensor_tensor(
                    mrstd_row, s0, 1.0 / E, rstd_row, op0=OP.mult, op1=OP.mult
                )  # mean * rstd
                for pt in range(EC):
                    grow, ngrow = rows_lpt[pt]
                    for c2 in range(CH // 2):
                        cs2 = slice(c2 * 1024, (c2 + 1) * 1024)
                        rb = ps_s.tile([P, 1024], F32, name="rb", tag="s")
                        cb = ps_s.tile([P, 1024], F32, name="cb", tag="s")
                        for h in range(2):
                            cs = slice(c2 * 1024 + h * 512, c2 * 1024 + (h + 1) * 512)
                            hs = slice(h * 512, (h + 1) * 512)
                            nc.tensor.matmul(rb[:, hs], grow[:], rstd_row[:, cs])
                            nc.tensor.matmul(cb[:, hs], ngrow[:], mrstd_row[:, cs])
                        t1 = tpool.tile([P, 1024], F16, name="t1", tag="t1")
                        nc.vector.tensor_mul(t1[:], X[pt][:, cs2], rb[:])
                        # X = (t1 + be) + cb  = x*g*rstd - g*mean*rstd + be
                        nc.vector.scalar_tensor_tensor(
                            X[pt][:, cs2], t1[:], be_col[:, pt : pt + 1], cb[:],
                            op0=OP.add, op1=OP.add,
                        )

            # ---- per batch element, layers interleaved across the two batch
            # elems so one stream's LN row math overlaps the other's matmuls --
            def input_proj(b):
                xs_t = xs_pool.tile([P, N], F16, name="xs", tag="xs")
                nc.sync.dma_start(xs_t[:], d_xsT[b])
                for ec in range(EC):
                    es = slice(ec * P, (ec + 1) * P)
                    for c in range(CH):
                        cs = slice(c * 512, (c + 1) * 512)
                        ps = ps_mm.tile([P, 512], F32, name="psin", tag="mm")
                        nc.tensor.matmul(ps[:], win_sb[:, es], xs_t[:, cs])
                        nc.vector.tensor_scalar_add(Hf[b][ec][:, cs], ps[:], binp_sb[:, ec : ec + 1])

            def qkv(b, l):
                for w_name, dstT in (("Wq", qT[b]), ("Wk", kT[b])):
                    for dc in range(EC):
                        ds_ = slice(dc * P, (dc + 1) * P)
                        for c in range(CH):
                            cs = slice(c * 512, (c + 1) * 512)
                            ps = ps_mm.tile([P, 512], F32, name="psqk", tag="mm")
                            for ec in range(EC):
                                nc.tensor.matmul(
                                    ps[:],
                                    w_sb[w_name][l][ec][:, ds_],
                                    Hf[b][ec][:, cs],
                                    start=(ec == 0),
                                    stop=(ec == EC - 1),
                                )
                            if w_name == "Wq":
                                nc.scalar.copy(dstT[dc][:, cs], ps[:])
                            else:
                                nc.vector.tensor_copy(dstT[dc][:, cs], ps[:])
                for t in range(JT):
                    ps = ps_mm.tile([P, E], F32, name="psv", tag="mm")
                    for ec in range(EC):
                        nc.tensor.matmul(
                            ps[:],
                            Hf[b][ec][:, t * P : (t + 1) * P],
                            w_sb["Wv"][l][ec][:],
                            start=(ec == 0),
                            stop=(ec == EC - 1),
                        )
                    if t % 2 == 0:
                        nc.scalar.copy(v_sb[b][:, t * E : (t + 1) * E], ps[:])
                    else:
                        nc.vector.tensor_copy(v_sb[b][:, t * E : (t + 1) * E], ps[:])

            def attention(b):
                for c in range(CH):
                    cs = slice(c * 512, (c + 1) * 512)
                    o_ps = [
                        ps_o.tile([P, 512], F32, name=f"o{oc}", tag="o")
                        for oc in range(EC)
                    ]
                    for j2 in range(JT // 2):
                        s_ps = ps_s.tile([P, 1024], F32, name="s_ps", tag="s")
                        for h in range(2):
                            j = 2 * j2 + h
                            hs = slice(h * 512, (h + 1) * 512)
                            for dc in range(EC):
                                nc.tensor.matmul(
                                    s_ps[:, hs],
                                    kT[b][dc][:, j * P : (j + 1) * P],
                                    qT[b][dc][:, cs],
                                    start=(dc == 0),
                                    stop=(dc == EC - 1),
                                )
                        sr = spool.tile([P, 1024], F16, name="sr", tag="sr")
                        nc.scalar.activation(sr[:], s_ps[:], AF.Relu)
                        for h in range(2):
                            j = 2 * j2 + h
                            hs = slice(h * 512, (h + 1) * 512)
                            for oc in range(EC):
                                nc.tensor.matmul(
                                    o_ps[oc][:],
                                    v_sb[b][:, j * E + oc * P : j * E + (oc + 1) * P],
                                    sr[:, hs],
                                    start=(j == 0),
                                    stop=(j == JT - 1),
                                )
                    for oc in range(EC):
                        nc.vector.tensor_add(Hf[b][oc][:, cs], Hf[b][oc][:, cs], o_ps[oc][:])

            def mlp(b, l):
                for c in range(CH):
                    cs = slice(c * 512, (c + 1) * 512)
                    a_t = []
                    for mc in range(EC):
                        ms = slice(mc * P, (mc + 1) * P)
                        ps = ps_mm.tile([P, 512], F32, name="psa", tag="mm")
                        for ec in range(EC):
                            nc.tensor.matmul(
                                ps[:],
                                w_sb["W1"][l][ec][:, ms],
                                Hf[b][ec][:, cs],
                                start=(ec == 0),
                                stop=(ec == EC - 1),
                            )
                        a = apool.tile([P, 512], F16, name="a", tag="a")
                        nc.scalar.activation(
                            a[:], ps[:], AF.Relu, bias=bm1_sb[l][:, mc : mc + 1]
                        )
                        a_t.append(a)
                    for oc in range(EC):
                        os_ = slice(oc * P, (oc + 1) * P)
                        ps = ps_mm.tile([P, 512], F32, name="psm", tag="mm")
                        for mc in range(EC):
                            nc.tensor.matmul(
                                ps[:],
                                w_sb["W2"][l][mc][:, os_],
                                a_t[mc][:],
                                start=(mc == 0),
                                stop=(mc == EC - 1),
                            )
                        nc.vector.scalar_tensor_tensor(
                            Hf[b][oc][:, cs],
                            ps[:],
                            bm2_sb[l][:, oc : oc + 1],
                            Hf[b][oc][:, cs],
                            op0=OP.add,
                            op1=OP.add,
                        )

            def readout(b):
                r8 = ropool.tile([P, 2 * CH], F32, name="r8", tag="r8")
                idx = 0
                for ec in range(EC):
                    for c in range(CH):
                        cs = slice(c * 512, (c + 1) * 512)
                        wo = wopool.tile([P, 512], F32, name="wo", tag="wo")
                        nc.sync.dma_start(
                            wo[:], d_WoutT[ec * P : (ec + 1) * P, c * 512 : (c + 1) * 512]
                        )
                        ros = ropool.tile([P, 512], F32, name="ros", tag="ros")
                        nc.vector.tensor_mul(ros[:], Hf[b][ec][:, cs], wo[:])
                        nc.vector.reduce_sum(
                            r8[:, idx : idx + 1], ros[:], axis=mybir.AxisListType.X
                        )
                        idx += 1
                rsum = ropool.tile([P, 1], F32, name="rsum", tag="rsum")
                nc.vector.reduce_sum(rsum[:], r8[:], axis=mybir.AxisListType.X)
                ps = ps_mm.tile([1, 1], F32, name="psro", tag="mm")
                nc.tensor.matmul(ps[:], ones_kf[:], rsum[:])
                ob = ropool.tile([1, 1], F32, name="ob", tag="ob")
                nc.scalar.activation(ob[:], ps[:], AF.Identity, bias=bout_sb[:])
                nc.sync.dma_start(d_out[b : b + 1, :], ob[:])

            for b in range(BL):
                input_proj(b)
            for l in range(L):
                for b in range(BL):
                    qkv(b, l)
                    attention(b)
                    layernorm(Hf[b], lnrow_sb[1][l], be1_sb[l])
                    mlp(b, l)
                    layernorm(Hf[b], lnrow_sb[2][l], be2_sb[l])
            for b in range(BL):
                readout(b)

    nc.compile()
    return nc


def _prep_inputs(inputs):
    f = lambda x: np.asarray(x, np.float32)
    bf = lambda x: np.ascontiguousarray(np.asarray(x, np.float32).astype(NPF16))
    xs = f(inputs["xs"])
    xsT = np.ascontiguousarray(xs.transpose(0, 2, 1)).astype(NPF16)  # [B, D, N]
    WoutT = np.ascontiguousarray(f(inputs["Wout"]).reshape(N, E).T)  # [E, N]

    def cols(v, per_l):
        v = f(v)
        if per_l:
            return np.ascontiguousarray(v.reshape(L, EC, P).transpose(0, 2, 1))
        return np.ascontiguousarray(v.reshape(EC, P).T)

    def grows(g):
        g = f(g).reshape(L, EC, P)
        return np.ascontiguousarray(np.stack([g, -g], axis=2)).astype(NPF16)

    common = {
        "Win": bf(inputs["Win"]),
        "Wq": bf(inputs["Wq"]),
        "Wk": bf(inputs["Wk"]),
        "Wv": bf(inputs["Wv"]),
        "W1": bf(inputs["W1"]),
        "W2": bf(inputs["W2"]),
        "WoutT": WoutT,
        "binp": cols(inputs["b_in"], False),
        "bm1p": cols(inputs["bm1"], True),
        "bm2p": cols(inputs["bm2"], True),
        "be1p": cols(inputs["be1"], True),
        "be2p": cols(inputs["be2"], True),
        "ln1rows": grows(inputs["g1"]),
        "ln2rows": grows(inputs["g2"]),
        "b_out": f(inputs["b_out"]).reshape(1, 1),
    }
    in_maps = []
    for c in range(NCORES):
        m = dict(common)
        m["xsT"] = np.ascontiguousarray(xsT[c * BL : (c + 1) * BL])
        in_maps.append(m)
    return in_maps


def get_program():
    if "nc" not in _CACHE:
        _CACHE["nc"] = _build()
    return _CACHE["nc"]


def kernel(**inputs) -> np.ndarray:
    nc = get_program()
    in_maps = _prep_inputs(inputs)
    res = run_bass_kernel_spmd(nc, in_maps, list(range(NCORES)))
    out = np.concatenate([res.results[c]["out"] for c in range(NCORES)], axis=0)
    return out.astype(np.float32)


# revision 14
# speedup vs baseline: 1.9135x; 1.0849x over previous
"""Trainium2 Bass kernel for nn_EncoderTransformer_61194694033513.

Data-parallel over batch B=16 across 8 NeuronCores (2 batch elems per core).
Per core, the whole forward runs out of SBUF with activations stored
feature-major HT[e, tok] in fp16 (matmul operands must be 16-bit to stream at
1 column/cycle on the PE; fp32 matmul runs at 1/4 rate; fp16 carries 10
mantissa bits vs bf16's 7, and squares are pre-scaled by 1/64 to stay in
fp16 range). All matmul
accumulation is fp32 in PSUM. Attention is computed flash-style (S^T tiles of
[128 keys x 512 queries], relu, accumulated into O^T) so the [N,N] matrix is
never materialized. LayerNorm reductions over the feature (partition) axis go
through the PE with a ones lhsT into [1,512] PSUM rows; the per-token row math
runs in fp32 on partition 0, rstd is computed as exp(-0.5*ln(var+eps)) on the
scalar engine (one table set, no slow DVE reciprocal), and rstd / mean*rstd
rows are broadcast back over partitions with K=1 matmuls whose lhsT carries
g / -g. The apply is one tensor_tensor + one scalar_tensor_tensor per 128x512
block (beta rides in as the per-partition fp32 scalar).
"""

import sys

import numpy as np

for _p in (
    "/opt/trn_rl_repo",
    "/opt/pypackages",
    "/root/.axon_site",
    "/root/.axon_site/_ro/trn_rl_repo",
    "/root/.axon_site/_ro/pypackages",
):
    if _p not in sys.path:
        sys.path.append(_p)

import ml_dtypes  # noqa: E402

import concourse.bass as bass  # noqa: E402
import concourse.bacc as bacc  # noqa: E402
import concourse.mybir as mybir  # noqa: E402
from concourse import tile  # noqa: E402
from concourse.bass_utils import run_bass_kernel_spmd  # noqa: E402

B, N, D, E, L = 16, 2048, 128, 256, 3
NCORES = 8
BL = B // NCORES  # batch elems per core
P = 128
EC = E // P  # feature-dim partition chunks (2)
CH = N // 512  # 512-wide token chunks (4)
JT = N // P  # key tiles (16)
EPS = 1e-5
F32 = mybir.dt.float32
F16 = mybir.dt.float16
NPF16 = np.float16
AF = mybir.ActivationFunctionType
OP = mybir.AluOpType

_CACHE = {}


def _build():
    nc = bacc.Bacc("TRN2", target_bir_lowering=False, debug=False, num_devices=NCORES)

    d_xsT = nc.declare_dram_parameter("xsT", [BL, P, N], F16, isOutput=False)
    d_Win = nc.declare_dram_parameter("Win", [D, E], F16, isOutput=False)
    d_W = {
        nm: nc.declare_dram_parameter(nm, [L, E, E], F16, isOutput=False)
        for nm in ("Wq", "Wk", "Wv", "W1", "W2")
    }
    d_WoutT = nc.declare_dram_parameter("WoutT", [E, N], F32, isOutput=False)
    d_binp = nc.declare_dram_parameter("binp", [P, EC], F32, isOutput=False)
    d_bm1 = nc.declare_dram_parameter("bm1p", [L, P, EC], F32, isOutput=False)
    d_bm2 = nc.declare_dram_parameter("bm2p", [L, P, EC], F32, isOutput=False)
    d_be1 = nc.declare_dram_parameter("be1p", [L, P, EC], F32, isOutput=False)
    d_be2 = nc.declare_dram_parameter("be2p", [L, P, EC], F32, isOutput=False)
    # g rows for the LN broadcast matmuls: [L, EC, 2, P]; row 0 = g, row 1 = -g
    d_ln1 = nc.declare_dram_parameter("ln1rows", [L, EC, 2, P], F16, isOutput=False)
    d_ln2 = nc.declare_dram_parameter("ln2rows", [L, EC, 2, P], F16, isOutput=False)
    d_bout = nc.declare_dram_parameter("b_out", [1, 1], F32, isOutput=False)
    d_out = nc.declare_dram_parameter("out", [BL, 1], F32, isOutput=True)

    with tile.TileContext(nc) as tc:
        from contextlib import ExitStack

        with ExitStack() as ctx:
            cpool = ctx.enter_context(tc.tile_pool(name="const", bufs=1))
            hpool = ctx.enter_context(tc.tile_pool(name="acts", bufs=1))
            xs_pool = ctx.enter_context(tc.tile_pool(name="xs", bufs=2))
            spool = ctx.enter_context(tc.tile_pool(name="srelu", bufs=6))
            sqpool = ctx.enter_context(tc.tile_pool(name="sqp", bufs=6))
            apool = ctx.enter_context(tc.tile_pool(name="mlpa", bufs=4))
            tpool = ctx.enter_context(tc.tile_pool(name="t1p", bufs=4))
            ropool = ctx.enter_context(tc.tile_pool(name="ro", bufs=2))
            wopool = ctx.enter_context(tc.tile_pool(name="wo", bufs=2))

            PS = bass.MemorySpace.PSUM
            ps_s = ctx.enter_context(tc.tile_pool(name="ps_s", bufs=2, space=PS))
            ps_o = ctx.enter_context(tc.tile_pool(name="ps_o", bufs=2, space=PS))
            ps_mm = ctx.enter_context(tc.tile_pool(name="ps_mm", bufs=2, space=PS))

            # ---- constants / weights -------------------------------------
            w_sb = {}
            for nm in ("Wq", "Wk", "Wv", "W1", "W2"):
                w_sb[nm] = []
                for l in range(L):
                    tl = []
                    for ec in range(EC):
                        t = cpool.tile([P, E], F16, name=f"{nm}{l}{ec}", tag=f"{nm}{l}{ec}")
                        nc.sync.dma_start(t[:], d_W[nm][l, ec * P : (ec + 1) * P, :])
                        tl.append(t)
                    w_sb[nm].append(tl)
            win_sb = cpool.tile([P, E], F16, name="win", tag="win")
            nc.sync.dma_start(win_sb[:], d_Win[:])

            def col_param(dram, nm, per_l=True):
                out = []
                for l in range(L if per_l else 1):
                    t = cpool.tile([P, EC], F32, name=f"{nm}{l}", tag=f"{nm}{l}")
                    nc.sync.dma_start(t[:], dram[l] if per_l else dram[:])
                    out.append(t)
                return out

            binp_sb = col_param(d_binp, "binp", per_l=False)[0]
            bm1_sb = col_param(d_bm1, "bm1")
            bm2_sb = col_param(d_bm2, "bm2")
            be1_sb = col_param(d_be1, "be1")
            be2_sb = col_param(d_be2, "be2")

            # g / -g broadcast rows: [1,128] bf16 tiles per (ln, l, pt)
            lnrow_sb = {1: [], 2: []}
            for which, dram in ((1, d_ln1), (2, d_ln2)):
                for l in range(L):
                    per_pt = []
                    for pt in range(EC):
                        rows = []
                        for r in range(2):
                            t = cpool.tile(
                                [1, P], F16,
                                name=f"ln{which}_{l}{pt}{r}", tag=f"ln{which}_{l}{pt}{r}",
                            )
                            nc.sync.dma_start(t[:], dram[l, pt, r : r + 1, :])
                            rows.append(t)
                        per_pt.append(rows)
                    lnrow_sb[which].append(per_pt)
            bout_sb = cpool.tile([1, 1], F32, name="bout", tag="bout")
            nc.sync.dma_start(bout_sb[:], d_bout[:])

            ones_kb = cpool.tile([P, 1], F16, name="ones_kb", tag="ones_kb")
            nc.vector.memset(ones_kb[:], 1.0)
            ones_kf = cpool.tile([P, 1], F32, name="ones_kf", tag="ones_kf")
            nc.vector.memset(ones_kf[:], 1.0)
            eps1 = cpool.tile([1, 1], F32, name="eps1", tag="eps1")
            nc.vector.memset(eps1[:], EPS)

            # LN row scratch: partition 0. rowsF fp32 (sum / sumsq / var),
            # rowsB bf16 (rstd / mean*rstd) for the broadcast matmul rhs.
            rowsF = cpool.tile([1, 3 * N], F32, name="rowsF", tag="rowsF")
            rowsB = cpool.tile([1, 2 * N], F16, name="rowsB", tag="rowsB")
            s0 = rowsF[:, 0:N]
            s1 = rowsF[:, N : 2 * N]
            xr = rowsF[:, 2 * N : 3 * N]
            rstd_row = rowsB[:, 0:N]
            mrstd_row = rowsB[:, N : 2 * N]

            # ---- persistent activations (fp16), one set per batch elem ----
            Hf = [[hpool.tile([P, N], F16, name=f"Hf{b}{ec}", tag=f"Hf{b}{ec}") for ec in range(EC)] for b in range(BL)]
            qT = [[hpool.tile([P, N], F16, name=f"qT{b}{dc}", tag=f"qT{b}{dc}") for dc in range(EC)] for b in range(BL)]
            kT = [[hpool.tile([P, N], F16, name=f"kT{b}{dc}", tag=f"kT{b}{dc}") for dc in range(EC)] for b in range(BL)]
            v_sb = [hpool.tile([P, JT * E], F16, name=f"v{b}", tag=f"v{b}") for b in range(BL)]

            def layernorm(X, rows_lpt, be_col):
                """In-place LN over the feature axis of X (list of 2 [P,N] fp16
                tiles). rows_lpt[pt] = (g_row, negg_row); be_col[:, pt] = beta.
                Stats, row math, broadcast and apply are all chunked 512 wide so
                the whole LN pipelines and the PE never waits on a long serial
                row chain."""
                for c in range(CH):
                    cs = slice(c * 512, (c + 1) * 512)
                    sqc = []
                    for pt in range(EC):
                        sq = sqpool.tile([P, 512], F16, name="sq", tag="sq")
                        nc.scalar.activation(sq[:], X[pt][:, cs], AF.Square, scale=1.0 / 64)
                        sqc.append(sq)
                    st_s = ps_mm.tile([1, 512], F32, name="st_s", tag="mm")
                    nc.tensor.matmul(st_s[:], ones_kb[:], X[0][:, cs], start=True, stop=False)
                    nc.tensor.matmul(st_s[:], ones_kb[:], X[1][:, cs], start=False, stop=True)
                    st_q = ps_mm.tile([1, 512], F32, name="st_q", tag="mm")
                    nc.tensor.matmul(st_q[:], ones_kb[:], sqc[0][:], start=True, stop=False)
                    nc.tensor.matmul(st_q[:], ones_kb[:], sqc[1][:], start=False, stop=True)
                    # chunk row math on partition 0 (fp32): sum -> SBUF, then
                    # var/4096 = stq - s0^2/(E*4096), rstd, mean*rstd
                    s0c = rowsF[:, c * 512 : (c + 1) * 512]
                    xrc = rowsF[:, N + c * 512 : N + (c + 1) * 512]
                    nc.scalar.copy(s0c, st_s[:])
                    nc.vector.tensor_mul(xrc, s0c, s0c)
                    nc.vector.scalar_tensor_tensor(
                        xrc, xrc, -1.0 / (E * 4096.0), st_q[:], op0=OP.mult, op1=OP.add
                    )
                    nc.scalar.activation(
                        rstd_row[:, cs], xrc, AF.Abs_reciprocal_sqrt,
                        bias=eps1[:], scale=4096.0 / E,
                    )
                    nc.vector.scalar_tensor_tensor(
                        mrstd_row[:, cs], s0c, 1.0 / E, rstd_row[:, cs],
                        op0=OP.mult, op1=OP.mult,
                    )
                for pt in range(EC):
                    grow, ngrow = rows_lpt[pt]
                    for c2 in range(CH // 2):
                        cs2 = slice(c2 * 1024, (c2 + 1) * 1024)
                        rb = ps_s.tile([P, 1024], F32, name="rb", tag="s")
                        cb = ps_s.tile([P, 1024], F32, name="cb", tag="s")
                        for h in range(2):
                            cs = slice(c2 * 1024 + h * 512, c2 * 1024 + (h + 1) * 512)
                            hs = slice(h * 512, (h + 1) * 512)
                            nc.tensor.matmul(rb[:, hs], grow[:], rstd_row[:, cs])
                            nc.tensor.matmul(cb[:, hs], ngrow[:], mrstd_row[:, cs])
                        t1 = tpool.tile([P, 1024], F16, name="t1", tag="t1")
                        nc.vector.tensor_mul(t1[:], X[pt][:, cs2], rb[:])
                        # X = (t1 + be) + cb  = x*g*rstd - g*mean*rstd + be
                        nc.vector.scalar_tensor_tensor(
                            X[pt][:, cs2], t1[:], be_col[:, pt : pt + 1], cb[:],
                            op0=OP.add, op1=OP.add,
                        )

            # ---- per batch element, layers interleaved across the two batch
            # elems so one stream's LN row math overlaps the other's matmuls --
            def input_proj(b):
                xs_t = xs_pool.tile([P, N], F16, name="xs", tag="xs")
                nc.sync.dma_start(xs_t[:], d_xsT[b])
                for ec in range(EC):
                    es = slice(ec * P, (ec + 1) * P)
                    for c in range(CH):
                        cs = slice(c * 512, (c + 1) * 512)
                        ps = ps_mm.tile([P, 512], F32, name="psin", tag="mm")
                        nc.tensor.matmul(ps[:], win_sb[:, es], xs_t[:, cs])
                        nc.vector.tensor_scalar_add(Hf[b][ec][:, cs], ps[:], binp_sb[:, ec : ec + 1])

            def qkv(b, l):
                for w_name, dstT in (("Wq", qT[b]), ("Wk", kT[b])):
                    for dc in range(EC):
                        ds_ = slice(dc * P, (dc + 1) * P)
                        for c in range(CH):
                            cs = slice(c * 512, (c + 1) * 512)
                            ps = ps_mm.tile([P, 512], F32, name="psqk", tag="mm")
                            for ec in range(EC):
                                nc.tensor.matmul(
                                    ps[:],
                                    w_sb[w_name][l][ec][:, ds_],
                                    Hf[b][ec][:, cs],
                                    start=(ec == 0),
                                    stop=(ec == EC - 1),
                                )
                            if w_name == "Wq":
                                nc.scalar.copy(dstT[dc][:, cs], ps[:])
                            else:
                                nc.vector.tensor_copy(dstT[dc][:, cs], ps[:])
                for t in range(JT):
                    ps = ps_mm.tile([P, E], F32, name="psv", tag="mm")
                    for ec in range(EC):
                        nc.tensor.matmul(
                            ps[:],
                            Hf[b][ec][:, t * P : (t + 1) * P],
                            w_sb["Wv"][l][ec][:],
                            start=(ec == 0),
                            stop=(ec == EC - 1),
                        )
                    if t % 2 == 0:
                        nc.scalar.copy(v_sb[b][:, t * E : (t + 1) * E], ps[:])
                    else:
                        nc.vector.tensor_copy(v_sb[b][:, t * E : (t + 1) * E], ps[:])

            def attention(b):
                for c in range(CH):
                    cs = slice(c * 512, (c + 1) * 512)
                    o_ps = [
                        ps_o.tile([P, 512], F32, name=f"o{oc}", tag="o")
                        for oc in range(EC)
                    ]
                    for j2 in range(JT // 2):
                        s_ps = ps_s.tile([P, 1024], F32, name="s_ps", tag="s")
                        for h in range(2):
                            j = 2 * j2 + h
                            hs = slice(h * 512, (h + 1) * 512)
                            for dc in range(EC):
                                nc.tensor.matmul(
                                    s_ps[:, hs],
                                    kT[b][dc][:, j * P : (j + 1) * P],
                                    qT[b][dc][:, cs],
                                    start=(dc == 0),
                                    stop=(dc == EC - 1),
                                )
                        sr = spool.tile([P, 1024], F16, name="sr", tag="sr")
                        nc.scalar.activation(sr[:], s_ps[:], AF.Relu)
                        for h in range(2):
                            j = 2 * j2 + h
                            hs = slice(h * 512, (h + 1) * 512)
                            for oc in range(EC):
                                nc.tensor.matmul(
                                    o_ps[oc][:],
                                    v_sb[b][:, j * E + oc * P : j * E + (oc + 1) * P],
                                    sr[:, hs],
                                    start=(j == 0),
                                    stop=(j == JT - 1),
                                )
                    for oc in range(EC):
                        nc.vector.tensor_add(Hf[b][oc][:, cs], Hf[b][oc][:, cs], o_ps[oc][:])

            def mlp(b, l):
                for c in range(CH):
                    cs = slice(c * 512, (c + 1) * 512)
                    a_t = []
                    for mc in range(EC):
                        ms = slice(mc * P, (mc + 1) * P)
                        ps = ps_mm.tile([P, 512], F32, name="psa", tag="mm")
                        for ec in range(EC):
                            nc.tensor.matmul(
                                ps[:],
                                w_sb["W1"][l][ec][:, ms],
                                Hf[b][ec][:, cs],
                                start=(ec == 0),
                                stop=(ec == EC - 1),
                            )
                        a = apool.tile([P, 512], F16, name="a", tag="a")
                        nc.scalar.activation(
                            a[:], ps[:], AF.Relu, bias=bm1_sb[l][:, mc : mc + 1]
                        )
                        a_t.append(a)
                    for oc in range(EC):
                        os_ = slice(oc * P, (oc + 1) * P)
                        ps = ps_mm.tile([P, 512], F32, name="psm", tag="mm")
                        for mc in range(EC):
                            nc.tensor.matmul(
                                ps[:],
                                w_sb["W2"][l][mc][:, os_],
                                a_t[mc][:],
                                start=(mc == 0),
                                stop=(mc == EC - 1),
                            )
                        nc.vector.scalar_tensor_tensor(
                            Hf[b][oc][:, cs],
                            ps[:],
                            bm2_sb[l][:, oc : oc + 1],
                            Hf[b][oc][:, cs],
                            op0=OP.add,
                            op1=OP.add,
                        )

            def readout(b):
                r8 = ropool.tile([P, 2 * CH], F32, name="r8", tag="r8")
                idx = 0
                for ec in range(EC):
                    for c in range(CH):
                        cs = slice(c * 512, (c + 1) * 512)
                        wo = wopool.tile([P, 512], F32, name="wo", tag="wo")
                        nc.sync.dma_start(
                            wo[:], d_WoutT[ec * P : (ec + 1) * P, c * 512 : (c + 1) * 512]
                        )
                        ros = ropool.tile([P, 512], F32, name="ros", tag="ros")
                        nc.vector.tensor_mul(ros[:], Hf[b][ec][:, cs], wo[:])
                        nc.vector.reduce_sum(
                            r8[:, idx : idx + 1], ros[:], axis=mybir.AxisListType.X
                        )
                        idx += 1
                rsum = ropool.tile([P, 1], F32, name="rsum", tag="rsum")
                nc.vector.reduce_sum(rsum[:], r8[:], axis=mybir.AxisListType.X)
                ps = ps_mm.tile([1, 1], F32, name="psro", tag="mm")
                nc.tensor.matmul(ps[:], ones_kf[:], rsum[:])
                ob = ropool.tile([1, 1], F32, name="ob", tag="ob")
                nc.scalar.activation(ob[:], ps[:], AF.Identity, bias=bout_sb[:])
                nc.sync.dma_start(d_out[b : b + 1, :], ob[:])

            for b in range(BL):
                input_proj(b)
            for l in range(L):
                for b in range(BL):
                    qkv(b, l)
                    attention(b)
                    layernorm(Hf[b], lnrow_sb[1][l], be1_sb[l])
                    mlp(b, l)
                    layernorm(Hf[b], lnrow_sb[2][l], be2_sb[l])
            for b in range(BL):
                readout(b)

    nc.compile()
    return nc


def _prep_inputs(inputs):
    f = lambda x: np.asarray(x, np.float32)
    bf = lambda x: np.ascontiguousarray(np.asarray(x, np.float32).astype(NPF16))
    xs = f(inputs["xs"])
    xsT = np.ascontiguousarray(xs.transpose(0, 2, 1)).astype(NPF16)  # [B, D, N]
    WoutT = np.ascontiguousarray(f(inputs["Wout"]).reshape(N, E).T)  # [E, N]

    def cols(v, per_l):
        v = f(v)
        if per_l:
            return np.ascontiguousarray(v.reshape(L, EC, P).transpose(0, 2, 1))
        return np.ascontiguousarray(v.reshape(EC, P).T)

    def grows(g):
        g = f(g).reshape(L, EC, P)
        return np.ascontiguousarray(np.stack([g, -g], axis=2)).astype(NPF16)

    common = {
        "Win": bf(inputs["Win"]),
        "Wq": bf(inputs["Wq"]),
        "Wk": bf(inputs["Wk"]),
        "Wv": bf(inputs["Wv"]),
        "W1": bf(inputs["W1"]),
        "W2": bf(inputs["W2"]),
        "WoutT": WoutT,
        "binp": cols(inputs["b_in"], False),
        "bm1p": cols(inputs["bm1"], True),
        "bm2p": cols(inputs["bm2"], True),
        "be1p": cols(inputs["be1"], True),
        "be2p": cols(inputs["be2"], True),
        "ln1rows": grows(inputs["g1"]),
        "ln2rows": grows(inputs["g2"]),
        "b_out": f(inputs["b_out"]).reshape(1, 1),
    }
    in_maps = []
    for c in range(NCORES):
        m = dict(common)
        m["xsT"] = np.ascontiguousarray(xsT[c * BL : (c + 1) * BL])
        in_maps.append(m)
    return in_maps


def get_program():
    if "nc" not in _CACHE:
        _CACHE["nc"] = _build()
    return _CACHE["nc"]


def kernel(**inputs) -> np.ndarray:
    nc = get_program()
    in_maps = _prep_inputs(inputs)
    res = run_bass_kernel_spmd(nc, in_maps, list(range(NCORES)))
    out = np.concatenate([res.results[c]["out"] for c in range(NCORES)], axis=0)
    return out.astype(np.float32)
